# revision 88
# baseline (speedup 1.0000x reference)
"""Trainium2 Bass kernel for nn_DES_PSP_Model (LSTM encoder + CNN + AR decoder).

Sharding: pure data parallel, batch 128 -> 8 cores x 16.

Encoder: 5-layer LSTM over T=256 run as a time wavefront (tick s computes
cell (l, s-l) for all valid l) with cross-layer batched vector ops in
[4H -> partitions, 5 layers x 16 batch -> free] layout.

Cell math (all-tanh trick): store H=2h, C=2c. Host pre-scales weights:
g-gate rows x2, h-input columns x0.5, gate chunks permuted to
chunkA=[f;i], chunkB=[o;g]. One ACT tanh(0.5*psum) gives s=tanh of all
gates; sigma(x) = 0.5(s+1). Then
  m1 = (sf+1)*C ; m2 = (si+1)*sg ; C' = 0.5*m1 + m2
  tc = tanh(0.5*C') ; H' = (so+1)*tc
Biases enter the psum via a K=6 matmul: stationary [x-row; 5 bias rows],
rhs = [x_t broadcast-slot; one-hot layer indicators].

CNN: conv0+avgpool folded (host im2col of the 1-channel input, W0/4),
conv1-7 as 9 shifted-AP matmuls (fp32r) with 2-way PE row tiling over a
partition-duplicated activation tile; ReLU+bias on ACT; GAP on DVE.

Decoder: 14 sequential steps x 5 layers, same cell, single 16-batch
pipe (measured identical to a 2-pipe split -- the decoder is latency-
chain-bound, so fewer ops at the same chain length wins on simplicity).
Gate bias enters the psum via a K=2 matmul (const ones rhs) so ONE tanh
ACT covers both chunks; step 0's cells 0..3 are emitted interleaved
into the encoder's tail ticks; fc writeback on DVE.

Changes vs v2 (886us -> 858us measured):
- Encoder psum ring (KR=4): persistent psum tiles [128, L*KR*16] per
  gate chunk. Every KR ticks per layer, ONE K=65 batch matmul (rows =
  [0.5*Wih.T; bias], rhs = H-ring block of layer l-1 + const-ones row)
  pre-computes Wih@h + bias for 4 future steps off the critical path;
  layer 0 ditto via a K=2 [x-row; b0] matmul against the time-major x
  line. Per tick only the 10 serial Whh matmuls remain (queue 2.0 ->
  1.25us/tick), which lets the CNN's ~150us of PE work hide in the
  Tensor gaps instead of stretching the encoder. Kills the big rhsx
  one-hot tensor entirely.
- conv0+avgpool folded into ONE 16-tap host im2col (K=16 matmuls): no
  on-device pooling ops at all.
- CNN slices paced evenly over all 272 ticks at single-matmul /
  single-image granularity; conv1-6 relu+bias on Scalar ACT (Scalar is
  cold in the CNN phase, DVE is what stalls the encoder loop); the
  LAST conv layer's relu ACT uses accum_out so the global-avg-pool sum
  falls out for free (no DVE reductions at all).
- Decoder: fc bias-add/writeback on DVE (Scalar is the decoder's
  hottest queue); state Htb has two step-parity column banks.

Known hard bounds (measured): the encoder tick is serial-loop-bound at
~2.39us (Whh MMs -> tanh -> m2 -> C -> tanh -> H across 3 engines),
NOT queue-bound, so further matmul-count cuts alone do not help. PSUM
note: a matmul with start=True clears the has_written accumulation
bits of its ENTIRE psum bank (verified by microbenchmark), so two
accumulation groups must never be open in one bank across a start --
this killed a decoder pre-accumulation attempt and bounds the design
space for psum reuse. (The encoder ring is safe: each group's start
and all its stops complete before the next start to that bank region's
group... empirically correct at 4.7e-3.)
"""
import os
import sys
import numpy as np
from contextlib import ExitStack

sys.path.insert(0, "/opt/trn_rl_repo")
os.environ.setdefault("JAX_PLATFORMS", "axon")

import ml_dtypes  # noqa: E402

BF = ml_dtypes.bfloat16

B, T, HID, L, PS = 128, 256, 64, 5, 14
ALPHA = 0.2
CNN_LAYERS = 8
NCORES = 8
BP = B // NCORES          # 16 batch per core
G4 = 4 * HID              # 256
W5 = L * BP               # 80  (5 layer slots x 16 batch)
IMG = 32                  # input image side
PM = 16                   # pooled side
PPAD = PM + 2             # 18 padded side
PIMG = PPAD * PPAD        # 324 per padded image

# pytorch gate rows: i[0:64] f[64:128] g[128:192] o[192:256]
# chunkA rows = [f; i], chunkB rows = [o; g]
_PERM_A = np.r_[64:128, 0:64]
_PERM_B = np.r_[192:256, 128:192]


# ----------------------------------------------------------------------------
# host-side weight preparation (pure layout/scale transforms)
# ----------------------------------------------------------------------------

def _gate_row_scale():
    """Row scale in chunk-permuted order: g rows x2 (chunkB bottom half)."""
    sA = np.ones(128, np.float32)
    sB = np.ones(128, np.float32)
    sB[64:128] = 2.0
    return sA, sB


def _chunk(W, perm, rowscale):
    # W: [4H, K] -> permuted+scaled chunk [128, K]
    return W[perm] * rowscale[:, None]


def _stat_kstack(Wih, Whh, perm, rowscale):
    """lhsT [128,128] for layers>=1: rows 0:64 Wih-part (h-in, x0.5),
    rows 64:128 Whh-part (x0.5)."""
    ci = _chunk(Wih, perm, rowscale) * 0.5   # [128, 64]
    ch = _chunk(Whh, perm, rowscale) * 0.5   # [128, 64]
    return np.concatenate([ci.T, ch.T], axis=0)  # [128, 128]


def prep_host(inputs):
    """Build per-core input maps (list of dicts of np arrays)."""
    x = np.asarray(inputs["x"], np.float32)
    y = np.asarray(inputs["y"], np.float32)
    f32 = lambda a: np.asarray(a, np.float32)
    enc_Wih0, enc_Wih = f32(inputs["enc_Wih0"]), f32(inputs["enc_Wih"])
    enc_Whh, enc_b = f32(inputs["enc_Whh"]), f32(inputs["enc_b"])
    dec_Wih0, dec_Wih = f32(inputs["dec_Wih0"]), f32(inputs["dec_Wih"])
    dec_Whh, dec_b = f32(inputs["dec_Whh"]), f32(inputs["dec_b"])
    fc_W, fc_b = f32(inputs["fc_W"]), f32(inputs["fc_b"])
    conv0_W, conv0_b = f32(inputs["conv0_W"]), f32(inputs["conv0_b"])
    convs_W, convs_b = f32(inputs["convs_W"]), f32(inputs["convs_b"])

    sA, sB = _gate_row_scale()

    # ---- lstmw: bf16 [128, nblocks*128 + 128] ----
    # All recurrent stationaries are K=64 [64,128] blocks in rows 0:64 so
    # matmuls read H (a [64, *] tile) directly -- no K-stack, no shift copy.
    blocks = []  # list of [128, 128] blocks (f32)

    def k64(W, perm, rowscale):  # [64, 128] lhsT in rows 0:64 (H is 2h)
        blk_ = np.zeros((128, 128), np.float32)
        blk_[0:64] = (_chunk(W, perm, rowscale) * 0.5).T
        return blk_

    # encoder (psum-ring layout, KR=4 timesteps per Wih/bias batch):
    # 0/1: layer-0 x+bias batch blocks A/B (K=2: row0 = Wih0 col, row1 = b0)
    # 2..11: Whh_l A/B (K=64), l = 0..4 at 2+2l / 3+2l
    # 12..19: Wih+bias batch blocks A/B for l=1..4 (K=65: rows 0:64 =
    #         (0.5*Wih).T, row 64 = b_l; rhs row 64 is a const-ones line)
    def l0batch(perm, rowscale):
        blk_ = np.zeros((128, 128), np.float32)
        blk_[0] = _chunk(enc_Wih0, perm, rowscale)[:, 0]
        blk_[1] = _chunk(enc_b[0][:, None], perm, rowscale)[:, 0]
        return blk_

    def wihbatch(l, perm, rowscale):
        blk_ = np.zeros((128, 128), np.float32)
        blk_[0:64] = (_chunk(enc_Wih[l - 1], perm, rowscale) * 0.5).T
        blk_[64] = _chunk(enc_b[l][:, None], perm, rowscale)[:, 0]
        return blk_

    blocks += [l0batch(_PERM_A, sA), l0batch(_PERM_B, sB)]
    for l in range(L):
        blocks += [k64(enc_Whh[l], _PERM_A, sA), k64(enc_Whh[l], _PERM_B, sB)]
    for l in range(1, L):
        blocks += [wihbatch(l, _PERM_A, sA), wihbatch(l, _PERM_B, sB)]
    # decoder: 20/21 Wy (row 0, unscaled: y is not doubled), 22/23 l0 Whh,
    # per layer 1..4: WihA@(24+4(l-1)) WhhA WihB WhhB -> 24..39
    wyA = np.zeros((128, 128), np.float32)
    wyB = np.zeros((128, 128), np.float32)
    wyA[0] = _chunk(dec_Wih0, _PERM_A, sA)[:, 0]
    wyB[0] = _chunk(dec_Wih0, _PERM_B, sB)[:, 0]
    blocks += [wyA, wyB]

    def k64b(W, b, perm, rowscale):
        # dec Whh block with the gate bias folded into row 64 (the rhs
        # Htb carries a const-ones line at partition 64); bias is NOT
        # halved (it multiplies ones, not H=2h).
        blk_ = k64(W, perm, rowscale)
        blk_[64] = _chunk(b[:, None], perm, rowscale)[:, 0]
        return blk_

    blocks += [k64b(dec_Whh[0], dec_b[0], _PERM_A, sA),
               k64b(dec_Whh[0], dec_b[0], _PERM_B, sB)]
    for l in range(1, L):
        blocks += [k64(dec_Wih[l - 1], _PERM_A, sA),
                   k64b(dec_Whh[l], dec_b[l], _PERM_A, sA),
                   k64(dec_Wih[l - 1], _PERM_B, sB),
                   k64b(dec_Whh[l], dec_b[l], _PERM_B, sB)]
    # decoder bias blocks 40..44: rows 0:2 = [bA; bB] (in-psum scale, g x2)
    for l in range(L):
        bb = np.zeros((128, 128), np.float32)
        bb[0] = _chunk(dec_b[l][:, None], _PERM_A, sA)[:, 0]
        bb[1] = _chunk(dec_b[l][:, None], _PERM_B, sB)[:, 0]
        blocks.append(bb)
    lstmw = np.concatenate(blocks, axis=1)  # [128, 45*128]
    # fc block: col 45*128 holds lhsT [64,1] = (0.5*fc_W).T
    fccol = np.zeros((128, 64), np.float32)
    fccol[0:64, 0] = 0.5 * fc_W[0]
    # conv0(+avgpool) 16-tap stationary [16, 64]:
    # V[t=(ty,tx), ch] = 1/4 sum_{ry,rx in 0..1} W0[ch, ty-ry, tx-rx]
    c0 = np.zeros((128, 64), np.float32)
    for ty in range(4):
        for tx in range(4):
            acc = np.zeros(64, np.float32)
            for ry in range(2):
                for rx in range(2):
                    ky, kx = ty - ry, tx - rx
                    if 0 <= ky < 3 and 0 <= kx < 3:
                        acc += conv0_W[:, 0, ky, kx]
            c0[ty * 4 + tx] = acc / 4.0
    lstmw = np.concatenate([lstmw, fccol, c0], axis=1).astype(BF)  # [128, 5888]

    # ---- cnnw: bf16 [128, 7*6*64]: uniform K=128 tap-pair stationaries ----
    # block p 0-2: rows 0:64 = tap (dy=p-1, dx=-1), rows 64:128 = tap (dy, 0)
    # block p 3-5: rows 0:64 = tap (dy=p-4, dx=+1), rows 64:128 = 0
    # (rhs bottom half is z pre-shifted by +1 column)
    cb = []
    for i in range(CNN_LAYERS - 1):
        for p in range(6):
            blk = np.zeros((128, 64), np.float32)
            if p < 3:
                dy = p - 1
                blk[0:64] = convs_W[i, :, :, dy + 1, 0].T
                blk[64:128] = convs_W[i, :, :, dy + 1, 1].T
            else:
                dy = p - 4
                blk[0:64] = convs_W[i, :, :, dy + 1, 2].T
            cb.append(blk)
    cnnw = np.concatenate(cb, axis=1).astype(BF)  # [128, 2688]

    # ---- misc: f32 [128, 32] ----
    misc = np.zeros((128, 32), np.float32)
    # decoder ACT bias (post-scale): i,f,o: 0.5*b ; g: b   (chunk-permuted)
    half = np.ones(256, np.float32) * 0.5
    half[128:192] = 1.0  # g rows (pytorch order) get 1.0
    for l in range(L):
        bb = dec_b[l] * half
        misc[:, 2 * l] = bb[_PERM_A]
        misc[:, 2 * l + 1] = bb[_PERM_B]
    misc[0, 10] = fc_b[0]
    misc[0:64, 11] = conv0_b
    for i in range(CNN_LAYERS - 1):
        misc[0:64, 12 + i] = convs_b[i]

    # ---- per-core tensors ----
    ypad = np.pad(y[:, 0], ((0, 0), (1, 1), (1, 1)))  # [B, 34, 34]
    in_maps = []
    for c in range(NCORES):
        sl = slice(c * BP, (c + 1) * BP)
        xs = x[sl, :, 0]  # [BP, T]
        # x2t [2, T*16 + 32]: row0 = x (time-major) then decoder-ones row0,
        # row1 = const ones (bias rhs line) then decoder-ones row1; the
        # decoder ones pattern covers both pipes (A|B per 16-col half)
        x2t = np.zeros((2, T * BP + 4 * BP), np.float32)
        x2t[0, 0:T * BP] = np.ascontiguousarray(xs.T).reshape(T * BP)
        x2t[1, 0:T * BP] = 1.0
        for po in (0, 16):
            x2t[0, T * BP + po:T * BP + po + 8] = 1.0
            x2t[1, T * BP + po + 8:T * BP + po + 16] = 1.0
        # single-pipe decoder ones pattern: [2, 32] (A cols 0:16, B 16:32)
        x2t[0, T * BP + 32:T * BP + 48] = 1.0
        x2t[1, T * BP + 48:T * BP + 64] = 1.0
        x2t = x2t.astype(BF)
        # pooled-conv0 16-tap im2col [16, BP*256]
        yp = ypad[sl]  # [BP, 34, 34]
        yim = np.zeros((16, BP, PM, PM), np.float32)
        for ty in range(4):
            for tx in range(4):
                yim[ty * 4 + tx] = yp[:, ty:ty + 2 * PM:2, tx:tx + 2 * PM:2]
        yim = yim.reshape(16, BP * PM * PM).astype(BF)
        in_maps.append(dict(
            lstmw=lstmw, cnnw=cnnw, misc=misc,
            x=x2t, yim=yim,
        ))
    return in_maps


# ----------------------------------------------------------------------------
# device program
# ----------------------------------------------------------------------------

_CACHE = {}


def build_program():
    import concourse.bass as bass  # noqa: F401
    import concourse.tile as tile
    from concourse import bacc, mybir

    F32 = mybir.dt.float32
    F32R = mybir.dt.float32r
    BF16 = mybir.dt.bfloat16
    AF = mybir.ActivationFunctionType
    OP = mybir.AluOpType

    KR = 4                       # ring depth: timesteps per Wih/bias batch
    TICKS = T + (L - 1) * KR     # 272
    DSTEPS = int(os.environ.get("BASSK_DSTEPS", PS))
    # NOTE: decoder matmul pre-accumulation (DPIPE=1) is numerically broken:
    # a matmul with start=True clears the has_written accumulation bits for
    # its ENTIRE psum bank (verified on HW), wiping other layers' open
    # accumulation groups that share the bank. Keep 0.
    HSPLIT = int(os.environ.get("BASSK_HSPLIT", 0))
    RELU_ACT = int(os.environ.get("BASSK_RELU_ACT", 1))
    DO_CNN = int(os.environ.get("BASSK_CNN", 1))
    NCONV = int(os.environ.get("BASSK_NCONV", CNN_LAYERS))
    DO_GAP = int(os.environ.get("BASSK_GAP", 1))

    nc = bacc.Bacc("TRN2", target_bir_lowering=False, debug=False,
                   num_devices=NCORES)
    d_lstmw = nc.dram_tensor("lstmw", [128, 5888], BF16, kind="ExternalInput").ap()
    d_cnnw = nc.dram_tensor("cnnw", [128, 2688], BF16, kind="ExternalInput").ap()
    d_misc = nc.dram_tensor("misc", [128, 32], F32, kind="ExternalInput").ap()
    d_x = nc.dram_tensor("x", [2, T * BP + 4 * BP], BF16,
                         kind="ExternalInput").ap()
    d_yim = nc.dram_tensor("yim", [16, BP * PM * PM], BF16,
                           kind="ExternalInput").ap()
    d_out = nc.dram_tensor("out", [1, PS * BP], F32, kind="ExternalOutput").ap()

    # stationary block column offsets in lstmw
    def blk(i):
        return slice(i * 128, (i + 1) * 128)
    FC_COL = 45 * 128
    C0_COL = 45 * 128 + 64
    KB = KR * BP                 # 64: ring cols per layer

    with tile.TileContext(nc) as tc:
        with ExitStack() as ctx:
            const = ctx.enter_context(tc.tile_pool(name="const", bufs=1))
            state = ctx.enter_context(tc.tile_pool(name="state", bufs=1))
            spool = ctx.enter_context(tc.tile_pool(name="spool", bufs=2))
            mpool = ctx.enter_context(tc.tile_pool(name="mpool", bufs=2))
            dpool = ctx.enter_context(tc.tile_pool(name="dpool", bufs=2))
            rps = ctx.enter_context(tc.tile_pool(name="rps", bufs=1, space="PSUM"))
            cps = ctx.enter_context(tc.tile_pool(name="cps", bufs=2, space="PSUM"))
            dps = ctx.enter_context(tc.tile_pool(name="dps", bufs=1, space="PSUM"))

            # ---- constants ----
            # DMA order: tick-0-critical pieces first (x line + the 20
            # encoder weight blocks), then the rest of lw, then CNN data,
            # so the encoder wavefront starts ~4us earlier.
            xw = const.tile([2, T * BP + 4 * BP], BF16, tag="xw", name="xw")
            nc.sync.dma_start(xw[:], d_x)
            misct = const.tile([128, 32], F32, tag="misct", name="misct")
            nc.sync.dma_start(misct[:], d_misc)
            lw = const.tile([128, 5888], BF16, tag="lw", name="lw")
            ENC_W = 20 * 128
            nc.sync.dma_start(lw[:, 0:ENC_W], d_lstmw[:, 0:ENC_W])
            nc.sync.dma_start(lw[:, ENC_W:], d_lstmw[:, ENC_W:])
            cw = const.tile([128, 2688], BF16, tag="cw", name="cw") if DO_CNN else None
            if DO_CNN:
                nc.sync.dma_start(cw[:], d_cnnw)
            yimt = const.tile([16, BP * PM * PM], BF16, tag="yimt", name="yimt") if DO_CNN else None
            if DO_CNN:
                nc.sync.dma_start(yimt[:], d_yim)

            # ---- persistent state ----
            # Hring[0:64, l*KB + j*BP + b] = 2*h^l[b] at timestep t, j=t%KR
            # (a KR-deep history ring per layer; row 64 = const ones so the
            # K=65 Wih-batch matmuls add the layer bias for free)
            Hring = state.tile([65, L * KB], BF16, tag="H", name="H")
            # Htb: decoder-only latest-H, two step-parity column banks;
            # partition 64 is a const-ones line so the K=65 Whh matmuls
            # add the decoder gate bias for free
            Htb = state.tile([65, 2 * W5], BF16, tag="Hd", name="Hd")
            nc.gpsimd.memset(Htb[64:65, :], 1.0)
            Ct = state.tile([64, W5], F32, tag="C", name="C")
            ydata = state.tile([1, BP], BF16, tag="ydata", name="ydata")
            nc.gpsimd.memset(Hring[0:64, :], 0.0)
            nc.gpsimd.memset(Hring[64:65, :], 1.0)
            nc.gpsimd.memset(Ct[:], 0.0)
            z2a = state.tile([128, BP * PIMG], BF16, tag="z2a", name="z2a") if DO_CNN else None
            z2b = state.tile([128, BP * PIMG], BF16, tag="z2b", name="z2b") if DO_CNN else None
            if DO_CNN:
                nc.gpsimd.memset(z2a[:], 0.0)
                nc.gpsimd.memset(z2b[:], 0.0)
            feat = state.tile([64, BP], F32, tag="feat", name="feat")
            feat2 = state.tile([64, BP], BF16, tag="feat2", name="feat2")
            outt = state.tile([1, PS * BP], F32, tag="outt", name="outt")
            if DSTEPS == 0:
                nc.gpsimd.memset(outt[:], 0.0)

            # =============== CNN emission slices ===============
            # CNN ops are emitted interleaved into the encoder tick loop in
            # small slices so PE/Scalar/Vector FIFO insertions never stall
            # the encoder's serial chain by more than ~1 op.
            cnn_slices = []
            if DO_CNN:
                c0st = lw[:, C0_COL:C0_COL + 64]  # [9 rows used, 64]
                z1v = z2a[:].rearrange("p (i r c) -> p i r c", i=BP, r=PPAD)

                def conv0_chunk(n):  # 2 pooled images per chunk, n = 0..7
                    def emit():
                        pc = cps.tile([64, 512], F32, tag="cpg", name="cpg")
                        nc.tensor.matmul(
                            pc[:], c0st[0:16, :],
                            yimt[0:16, n * 512:(n + 1) * 512],
                            start=True, stop=True)
                        i0 = 2 * n
                        pcv = pc[:].rearrange("p (i r c) -> p i r c",
                                              i=2, r=PM)
                        nc.scalar.activation(
                            z1v[0:64, i0:i0 + 2, 1:17, 1:17], pcv,
                            AF.Identity, bias=misct[0:64, 11:12])
                        nc.gpsimd.tensor_copy(
                            z1v[64:128, i0:i0 + 2, 1:17, 0:16],
                            z1v[0:64, i0:i0 + 2, 1:17, 1:17])
                    return emit

                for n in range(BP // 2):
                    cnn_slices.append(conv0_chunk(n))

                ccell = {}

                def conv_mms(i, n, prange, zin):
                    def emit():
                        ziv = zin[:].rearrange("p (i r c) -> p i r c",
                                               i=BP, r=PPAD)
                        if prange[0] == 0:
                            ccell['pc'] = cps.tile([64, 512], F32, tag="cpg",
                                                   name="cpg")
                        pc = ccell['pc']
                        i0 = 2 * n
                        for p in prange:
                            dy = (p - 1) if p < 3 else (p - 4)
                            c0_ = 0 if p < 3 else 2
                            st_ = cw[:, (i - 1) * 384 + p * 64:
                                     (i - 1) * 384 + p * 64 + 64]
                            rhs = ziv[:, i0:i0 + 2, 1 + dy:17 + dy,
                                      c0_:c0_ + 16]
                            nc.tensor.matmul(pc[:], st_, rhs,
                                             start=(p == 0), stop=(p == 5))
                    return emit

                def conv_relu(i, n, half, zout):
                    def emit():
                        pc = ccell['pc']
                        im = 2 * n + half
                        zov = zout[:].rearrange("p (i r c) -> p i r c",
                                                i=BP, r=PPAD)
                        pcv = pc[:].rearrange("p (i r c) -> p i r c",
                                              i=2, r=16)[:, half]
                        if i < CNN_LAYERS - 1:
                            # relu on Scalar ACT (measured best; a 50/50
                            # Scalar/DVE alternation and all-DVE were both
                            # slower on this base)
                            if RELU_ACT:
                                nc.scalar.activation(
                                    zov[0:64, im, 1:17, 1:17], pcv,
                                    AF.Relu,
                                    bias=misct[0:64, 11 + i:12 + i])
                            else:
                                nc.vector.tensor_scalar(
                                    zov[0:64, im, 1:17, 1:17], pcv,
                                    misct[0:64, 11 + i:12 + i], 0.0,
                                    op0=OP.add, op1=OP.max)
                            nc.gpsimd.tensor_copy(
                                zov[64:128, im, 1:17, 0:16],
                                zov[0:64, im, 1:17, 1:17])
                        else:
                            # last conv: Scalar ACT whose accum_out side
                            # output IS the global-avg-pool sum -- removes
                            # the relu AND the GAP reduction from DVE
                            nc.scalar.activation(
                                zov[0:64, im, 1:17, 1:17], pcv,
                                AF.Relu, bias=misct[0:64, 11 + i:12 + i],
                                accum_out=feat[:, im:im + 1])
                    return emit

                zin, zout = z2a, z2b
                for i in range(1, NCONV):
                    for n in range(BP // 2):
                        for p in range(6):
                            cnn_slices.append(conv_mms(i, n, (p,), zin))
                        cnn_slices.append(conv_relu(i, n, 0, zout))
                        cnn_slices.append(conv_relu(i, n, 1, zout))
                    zin, zout = zout, zin

                if DO_GAP:
                    # GAP already accumulated by the last conv layer's ACT
                    # accum_out; just cast to bf16 for the fuse
                    cnn_slices.append(
                        lambda: nc.vector.tensor_copy(feat2[:], feat[:]))
                else:
                    cnn_slices.append(
                        lambda: nc.gpsimd.memset(feat2[:], 0.0))
            else:
                cnn_slices.append(lambda: nc.gpsimd.memset(feat2[:], 0.0))

            # =============== decoder emission closures ===============
            # Defined up front so step 0's cells 0..3 can be emitted
            # INTERLEAVED into the encoder's tail ticks (layer l's final
            # encoder tick is 255+KR*l, so fuse-l and decoder cell l can
            # start while upper layers are still finishing).
            kf = 2.0 * ALPHA / 256.0
            jfin = (T - 1) % KR
            HB = BP // 2
            ones8 = xw[0:2, T * BP:T * BP + 2 * HB]

            def emit_fuse(l):
                # initial decoder state -> parity-1 columns of Htb
                nc.vector.scalar_tensor_tensor(
                    Htb[0:64, W5 + l * BP:W5 + (l + 1) * BP], feat2[:], kf,
                    Hring[0:64, l * KB + jfin * BP:l * KB + (jfin + 1) * BP],
                    op0=OP.mult, op1=OP.add)
                if l == 0:
                    nc.vector.tensor_copy(ydata[0:1, :],
                                          xw[0:1, (T - 1) * BP:T * BP])

            ones16 = xw[0:2, T * BP + 32:T * BP + 64]

            def emit_cell(step, l):
                # single 16-batch pipe: half the ops of the 2-pipe form
                # (1 gate ACT, 3 STT, 1 tc, 1 H per cell) -- bets queue
                # relief over pipe overlap (Scalar was ~70% busy)
                par = step % 2
                hp = (step - 1) % 2
                whA = blk(22) if l == 0 else blk(24 + 4 * (l - 1) + 1)
                whB = blk(23) if l == 0 else blk(24 + 4 * (l - 1) + 3)
                ch = hp * W5 + l * BP
                if l == 0 and 'pd' in pdz:
                    # off-path group pre-emitted before the previous fc
                    pd = pdz.pop('pd')
                else:
                    pd = dps.tile([128, 2 * BP], F32, tag="dpg0",
                                  name="dpg0")
                    nc.tensor.matmul(pd[:, 0:2 * BP], lw[0:2, blk(40 + l)],
                                     ones16, start=True, stop=False)
                    nc.tensor.matmul(pd[:, 0:BP], lw[0:64, whA],
                                     Htb[0:64, ch:ch + BP],
                                     start=False, stop=False)
                    nc.tensor.matmul(pd[:, BP:2 * BP], lw[0:64, whB],
                                     Htb[0:64, ch:ch + BP],
                                     start=False, stop=False)
                if l == 0:
                    nc.tensor.matmul(pd[:, 0:BP], lw[0:1, blk(20)],
                                     ydata[0:1, :], start=False, stop=True)
                    nc.tensor.matmul(pd[:, BP:2 * BP], lw[0:1, blk(21)],
                                     ydata[0:1, :], start=False, stop=True)
                else:
                    b0 = 24 + 4 * (l - 1)
                    p0 = par * W5 + (l - 1) * BP
                    nc.tensor.matmul(pd[:, 0:BP], lw[0:64, blk(b0)],
                                     Htb[0:64, p0:p0 + BP],
                                     start=False, stop=True)
                    nc.tensor.matmul(pd[:, BP:2 * BP], lw[0:64, blk(b0 + 2)],
                                     Htb[0:64, p0:p0 + BP],
                                     start=False, stop=True)
                sd = dpool.tile([128, 2 * BP], BF16, tag="sdec0",
                                name="sdec0")
                nc.scalar.activation(sd[:], pd[:], AF.Tanh, scale=0.5)
                c0 = l * BP
                dm1 = mpool.tile([64, BP], F32, tag="dm10", name="dm10")
                dm2 = mpool.tile([64, BP], BF16, tag="dm20", name="dm20")
                nc.vector.scalar_tensor_tensor(
                    dm1[:], sd[0:64, 0:BP], 1.0, Ct[:, c0:c0 + BP],
                    op0=OP.add, op1=OP.mult)
                nc.vector.scalar_tensor_tensor(
                    dm2[:], sd[64:128, 0:BP], 1.0, sd[64:128, BP:2 * BP],
                    op0=OP.add, op1=OP.mult)
                nc.vector.scalar_tensor_tensor(
                    Ct[:, c0:c0 + BP], dm1[:], 0.5, dm2[:],
                    op0=OP.mult, op1=OP.add)
                dtc = mpool.tile([64, BP], BF16, tag="dtc0", name="dtc0")
                nc.scalar.activation(dtc[:], Ct[:, c0:c0 + BP],
                                     AF.Tanh, scale=0.5)
                nc.vector.scalar_tensor_tensor(
                    Htb[0:64, par * W5 + c0:par * W5 + c0 + BP],
                    sd[0:64, BP:2 * BP], 1.0, dtc[:],
                    op0=OP.add, op1=OP.mult)

            pdz = {}

            def emit_fc(step):
                par = step % 2
                if step + 1 < DSTEPS:
                    # next step's cell-0 bias+Whh: inputs are ready (h^0 of
                    # this step), dedicated bank dpgZ has no open group
                    pdn = dps.tile([128, 2 * BP], F32, tag="dpgZ",
                                   name="dpgZ")
                    chn = par * W5
                    nc.tensor.matmul(pdn[:, 0:2 * BP], lw[0:2, blk(40)],
                                     ones16, start=True, stop=False)
                    nc.tensor.matmul(pdn[:, 0:BP], lw[0:64, blk(22)],
                                     Htb[0:64, chn:chn + BP],
                                     start=False, stop=False)
                    nc.tensor.matmul(pdn[:, BP:2 * BP], lw[0:64, blk(23)],
                                     Htb[0:64, chn:chn + BP],
                                     start=False, stop=False)
                    pdz['pd'] = pdn
                pf = dps.tile([128, BP], F32, tag="pfc0", name="pfc0")
                nc.tensor.matmul(
                    pf[0:1, 0:BP], lw[0:64, FC_COL:FC_COL + 1],
                    Htb[0:64, par * W5 + (L - 1) * BP:par * W5 + L * BP],
                    start=True, stop=True)
                if step + 1 < DSTEPS:
                    nc.vector.tensor_scalar(
                        ydata[0:1, :], pf[0:1, 0:BP],
                        misct[0:1, 10:11], 0.0, op0=OP.add, op1=OP.add)
                nc.vector.tensor_scalar(
                    outt[0:1, step * BP:(step + 1) * BP], pf[0:1, 0:BP],
                    misct[0:1, 10:11], 0.0, op0=OP.add, op1=OP.add)

            # =============== encoder wavefront (psum ring) ===============
            # Persistent psum rings [128 gates, L layers x KR slots x BP]:
            # every KR ticks per layer, one K=65 batch matmul (start=True)
            # writes Wih@h^{l-1} + bias for KR future steps; each tick only
            # the serial Whh matmul (stop=True, slot t%KR) is on the chain.
            pgA = rps.tile([128, L * KB], F32, tag="pgA", name="pgA")
            pgB = rps.tile([128, L * KB], F32, tag="pgB", name="pgB")
            pgA4 = pgA[:].rearrange("p (l j b) -> p l j b", l=L, j=KR)
            pgB4 = pgB[:].rearrange("p (l j b) -> p l j b", l=L, j=KR)
            Hr4 = Hring[0:64, :].rearrange("p (l j b) -> p l j b", l=L, j=KR)

            cnn_emitted = 0
            for s in range(TICKS):
                lmax = min(L - 1, s // KR)
                lmin = max(0, -((-(s - (T - 1))) // KR))
                lo, w = lmin * BP, (lmax - lmin + 1) * BP
                nl = lmax - lmin + 1
                j = s % KR
                jp = (s - 1) % KR

                # group-start batch MMs (Wih+bias over KR steps, off-chain)
                for l in range(lmin, lmax + 1):
                    t = s - KR * l
                    if t % KR == 0:
                        for c, pgc in ((0, pgA), (1, pgB)):
                            if l == 0:
                                nc.tensor.matmul(
                                    pgc[:, 0:KB], lw[0:2, blk(c)],
                                    xw[0:2, t * BP:t * BP + KB],
                                    start=True, stop=False)
                            else:
                                nc.tensor.matmul(
                                    pgc[:, l * KB:(l + 1) * KB],
                                    lw[0:65, blk(12 + 2 * (l - 1) + c)],
                                    Hring[0:65, (l - 1) * KB:l * KB],
                                    start=True, stop=False)
                # per-tick serial Whh MMs (slot j), chunk A then B
                for c, pgc in ((0, pgA), (1, pgB)):
                    for l in range(lmin, lmax + 1):
                        nc.tensor.matmul(
                            pgc[:, l * KB + j * BP:l * KB + (j + 1) * BP],
                            lw[0:64, blk(2 + 2 * l + c)],
                            Hring[0:64, l * KB + jp * BP:l * KB + (jp + 1) * BP],
                            start=False, stop=True)

                # gates: per-chunk tanh so chunk-A consumers start earlier
                stA = spool.tile([128, W5], BF16, tag="stA", name="stA")
                stB = spool.tile([128, W5], BF16, tag="stB", name="stB")
                sAv = stA[:, lo:lo + w].rearrange("p (l b) -> p l b", b=BP)
                sBv = stB[:, lo:lo + w].rearrange("p (l b) -> p l b", b=BP)
                nc.scalar.activation(sAv, pgA4[:, lmin:lmax + 1, j, :],
                                     AF.Tanh, scale=0.5)
                nc.scalar.activation(sBv, pgB4[:, lmin:lmax + 1, j, :],
                                     AF.Tanh, scale=0.5)

                m1 = mpool.tile([64, W5], BF16, tag="m1", name="m1")
                m2 = mpool.tile([64, W5], BF16, tag="m2", name="m2")
                tcn = mpool.tile([64, W5], BF16, tag="tc", name="tc")
                # m1 = (sf+1)*C  (needs only chunk A; runs under ACT-B)
                nc.vector.scalar_tensor_tensor(
                    m1[:, lo:lo + w], stA[0:64, lo:lo + w], 1.0,
                    Ct[:, lo:lo + w], op0=OP.add, op1=OP.mult)
                # m2 = (si+1)*sg
                nc.vector.scalar_tensor_tensor(
                    m2[:, lo:lo + w], stA[64:128, lo:lo + w], 1.0,
                    stB[64:128, lo:lo + w], op0=OP.add, op1=OP.mult)
                # C = 0.5*m1 + m2
                nc.vector.scalar_tensor_tensor(
                    Ct[:, lo:lo + w], m1[:, lo:lo + w], 0.5,
                    m2[:, lo:lo + w], op0=OP.mult, op1=OP.add)
                # tc = tanh(0.5*C)
                nc.scalar.activation(tcn[:, lo:lo + w], Ct[:, lo:lo + w],
                                     AF.Tanh, scale=0.5)
                # H = (so+1)*tc -> ring slot j of each active layer.
                # Split low/high layers so the next tick's low-layer Whh
                # matmuls can issue ~200ns earlier (the loop is the bound).
                lsp = min(lmax, max(lmin, 2)) if HSPLIT else lmax
                for (g0, g1) in (((lmin, lsp)), ((lsp + 1, lmax))):
                    if g1 < g0:
                        continue
                    c0_, c1_ = g0 * BP, (g1 + 1) * BP
                    nc.vector.scalar_tensor_tensor(
                        Hr4[:, g0:g1 + 1, j, :],
                        stB[0:64, c0_:c1_].rearrange("p (l b) -> p l b", b=BP),
                        1.0,
                        tcn[:, c0_:c1_].rearrange("p (l b) -> p l b", b=BP),
                        op0=OP.add, op1=OP.mult)

                # pace CNN slices so they finish by tick ~250 (feat2 must
                # be ready before fuse-l0 fires at tick 255), queued behind
                # this tick's ops to land in the idle windows
                target = min(len(cnn_slices),
                             (s + 1) * len(cnn_slices) // min(TICKS, 250))
                while cnn_emitted < target:
                    cnn_slices[cnn_emitted]()
                    cnn_emitted += 1

                # overlap decoder start with the encoder tail: fuse-l right
                # after layer l's last tick; step-0 cell l one tick later
                if DSTEPS > 0 and s >= T - 1 and (s - (T - 1)) % KR == 0:
                    lf = (s - (T - 1)) // KR
                    if lf < L:
                        emit_fuse(lf)
                if DSTEPS > 0 and s >= T and (s - T) % KR == 0:
                    lc = (s - T) // KR
                    if lc < L - 1:
                        emit_cell(0, lc)

            # leftover CNN slices (if any) after the encoder
            for k in range(cnn_emitted, len(cnn_slices)):
                cnn_slices[k]()

            # =============== decoder (two half-batch pipes) ===============
            # Two independent 8-batch chains interleave on the engines so
            # each pipe's serial latency hides under the other's work.
            # Step 0 cells 0..L-2 were already emitted in the tail ticks.
            for step in range(DSTEPS):
                for l in range(L - 1 if step == 0 else 0, L):
                    emit_cell(step, l)
                emit_fc(step)

            nc.sync.dma_start(d_out, outt[:])

    nc.compile()
    return nc


def kernel(**inputs) -> np.ndarray:
    from concourse.bass_utils import run_bass_kernel_spmd
    if "nc" not in _CACHE:
        _CACHE["nc"] = build_program()
    nc = _CACHE["nc"]
    in_maps = prep_host(inputs)
    res = run_bass_kernel_spmd(nc, in_maps, list(range(NCORES)))
    outs = []
    for c in range(NCORES):
        o = np.asarray(res.results[c]["out"], np.float32).reshape(PS, BP)
        outs.append(o.T[:, :, None])  # [BP, PS, 1]
    return np.concatenate(outs, axis=0)



# revision 89
# speedup vs baseline: 1.1962x; 1.1962x over previous
"""Trainium2 Bass kernel for nn_DES_PSP_Model (LSTM encoder + CNN + AR decoder).

Sharding: pure data parallel, batch 128 -> 8 cores x 16.

Encoder: 5-layer LSTM over T=256 run as a time wavefront (tick s computes
cell (l, s-l) for all valid l) with cross-layer batched vector ops in
[4H -> partitions, 5 layers x 16 batch -> free] layout.

Cell math (all-tanh trick): store H=2h, C=2c. Host pre-scales weights:
g-gate rows x2, h-input columns x0.5, gate chunks permuted to
chunkA=[f;i], chunkB=[o;g]. One ACT tanh(0.5*psum) gives s=tanh of all
gates; sigma(x) = 0.5(s+1). Then
  m1 = (sf+1)*C ; m2 = (si+1)*sg ; C' = 0.5*m1 + m2
  tc = tanh(0.5*C') ; H' = (so+1)*tc
Biases enter the psum via a K=6 matmul: stationary [x-row; 5 bias rows],
rhs = [x_t broadcast-slot; one-hot layer indicators].

CNN: conv0+avgpool folded (host im2col of the 1-channel input, W0/4),
conv1-7 as 9 shifted-AP matmuls (fp32r) with 2-way PE row tiling over a
partition-duplicated activation tile; ReLU+bias on ACT; GAP on DVE.

Decoder: 14 sequential steps x 5 layers, same cell, single 16-batch
pipe (measured identical to a 2-pipe split -- the decoder is latency-
chain-bound, so fewer ops at the same chain length wins on simplicity).
Gate bias enters the psum via a K=2 matmul (const ones rhs) so ONE tanh
ACT covers both chunks; step 0's cells 0..3 are emitted interleaved
into the encoder's tail ticks; fc writeback on DVE.

Changes vs v2 (886us -> 858us measured):
- Encoder psum ring (KR=4): persistent psum tiles [128, L*KR*16] per
  gate chunk. Every KR ticks per layer, ONE K=65 batch matmul (rows =
  [0.5*Wih.T; bias], rhs = H-ring block of layer l-1 + const-ones row)
  pre-computes Wih@h + bias for 4 future steps off the critical path;
  layer 0 ditto via a K=2 [x-row; b0] matmul against the time-major x
  line. Per tick only the 10 serial Whh matmuls remain (queue 2.0 ->
  1.25us/tick), which lets the CNN's ~150us of PE work hide in the
  Tensor gaps instead of stretching the encoder. Kills the big rhsx
  one-hot tensor entirely.
- conv0+avgpool folded into ONE 16-tap host im2col (K=16 matmuls): no
  on-device pooling ops at all.
- CNN slices paced evenly over all 272 ticks at single-matmul /
  single-image granularity; conv1-6 relu+bias on Scalar ACT (Scalar is
  cold in the CNN phase, DVE is what stalls the encoder loop); the
  LAST conv layer's relu ACT uses accum_out so the global-avg-pool sum
  falls out for free (no DVE reductions at all).
- Decoder: fc bias-add/writeback on DVE (Scalar is the decoder's
  hottest queue); state Htb has two step-parity column banks.

Known hard bounds (measured): the encoder tick is serial-loop-bound at
~2.39us (Whh MMs -> tanh -> m2 -> C -> tanh -> H across 3 engines),
NOT queue-bound, so further matmul-count cuts alone do not help. PSUM
note: a matmul with start=True clears the has_written accumulation
bits of its ENTIRE psum bank (verified by microbenchmark), so two
accumulation groups must never be open in one bank across a start --
this killed a decoder pre-accumulation attempt and bounds the design
space for psum reuse. (The encoder ring is safe: each group's start
and all its stops complete before the next start to that bank region's
group... empirically correct at 4.7e-3.)
"""
import os
import sys
import numpy as np
from contextlib import ExitStack

sys.path.insert(0, "/opt/trn_rl_repo")
os.environ.setdefault("JAX_PLATFORMS", "axon")

import ml_dtypes  # noqa: E402

BF = ml_dtypes.bfloat16

B, T, HID, L, PS = 128, 256, 64, 5, 14
ALPHA = 0.2
CNN_LAYERS = 8
NCORES = 8
BP = B // NCORES          # 16 batch per core
G4 = 4 * HID              # 256
W5 = L * BP               # 80  (5 layer slots x 16 batch)
IMG = 32                  # input image side
PM = 16                   # pooled side
PPAD = PM + 2             # 18 padded side
PIMG = PPAD * PPAD        # 324 per padded image

# pytorch gate rows: i[0:64] f[64:128] g[128:192] o[192:256]
# chunkA rows = [f; i], chunkB rows = [o; g]
_PERM_A = np.r_[64:128, 0:64]
_PERM_B = np.r_[192:256, 128:192]


# ----------------------------------------------------------------------------
# host-side weight preparation (pure layout/scale transforms)
# ----------------------------------------------------------------------------

def _gate_row_scale():
    """Row scale in chunk-permuted order: g rows x2 (chunkB bottom half)."""
    sA = np.ones(128, np.float32)
    sB = np.ones(128, np.float32)
    sB[64:128] = 2.0
    return sA, sB


def _chunk(W, perm, rowscale):
    # W: [4H, K] -> permuted+scaled chunk [128, K]
    return W[perm] * rowscale[:, None]


def _stat_kstack(Wih, Whh, perm, rowscale):
    """lhsT [128,128] for layers>=1: rows 0:64 Wih-part (h-in, x0.5),
    rows 64:128 Whh-part (x0.5)."""
    ci = _chunk(Wih, perm, rowscale) * 0.5   # [128, 64]
    ch = _chunk(Whh, perm, rowscale) * 0.5   # [128, 64]
    return np.concatenate([ci.T, ch.T], axis=0)  # [128, 128]


def prep_host(inputs):
    """Build per-core input maps (list of dicts of np arrays)."""
    x = np.asarray(inputs["x"], np.float32)
    y = np.asarray(inputs["y"], np.float32)
    f32 = lambda a: np.asarray(a, np.float32)
    enc_Wih0, enc_Wih = f32(inputs["enc_Wih0"]), f32(inputs["enc_Wih"])
    enc_Whh, enc_b = f32(inputs["enc_Whh"]), f32(inputs["enc_b"])
    dec_Wih0, dec_Wih = f32(inputs["dec_Wih0"]), f32(inputs["dec_Wih"])
    dec_Whh, dec_b = f32(inputs["dec_Whh"]), f32(inputs["dec_b"])
    fc_W, fc_b = f32(inputs["fc_W"]), f32(inputs["fc_b"])
    conv0_W, conv0_b = f32(inputs["conv0_W"]), f32(inputs["conv0_b"])
    convs_W, convs_b = f32(inputs["convs_W"]), f32(inputs["convs_b"])

    sA, sB = _gate_row_scale()

    # ---- lstmw: bf16 [128, nblocks*128 + 128] ----
    # All recurrent stationaries are K=64 [64,128] blocks in rows 0:64 so
    # matmuls read H (a [64, *] tile) directly -- no K-stack, no shift copy.
    blocks = []  # list of [128, 128] blocks (f32)

    def k64(W, perm, rowscale):  # [64, 128] lhsT in rows 0:64 (H is 2h)
        blk_ = np.zeros((128, 128), np.float32)
        blk_[0:64] = (_chunk(W, perm, rowscale) * 0.5).T
        return blk_

    # encoder (psum-ring layout, KR=4 timesteps per Wih/bias batch):
    # 0/1: layer-0 x+bias batch blocks A/B (K=2: row0 = Wih0 col, row1 = b0)
    # 2..11: Whh_l A/B (K=64), l = 0..4 at 2+2l / 3+2l
    # 12..19: Wih+bias batch blocks A/B for l=1..4 (K=65: rows 0:64 =
    #         (0.5*Wih).T, row 64 = b_l; rhs row 64 is a const-ones line)
    def l0batch(perm, rowscale):
        blk_ = np.zeros((128, 128), np.float32)
        blk_[0] = _chunk(enc_Wih0, perm, rowscale)[:, 0]
        blk_[1] = _chunk(enc_b[0][:, None], perm, rowscale)[:, 0]
        return blk_

    def wihbatch(l, perm, rowscale):
        blk_ = np.zeros((128, 128), np.float32)
        blk_[0:64] = (_chunk(enc_Wih[l - 1], perm, rowscale) * 0.5).T
        blk_[64] = _chunk(enc_b[l][:, None], perm, rowscale)[:, 0]
        return blk_

    blocks += [l0batch(_PERM_A, sA), l0batch(_PERM_B, sB)]
    for l in range(L):
        blocks += [k64(enc_Whh[l], _PERM_A, sA), k64(enc_Whh[l], _PERM_B, sB)]
    for l in range(1, L):
        blocks += [wihbatch(l, _PERM_A, sA), wihbatch(l, _PERM_B, sB)]
    # decoder: 20/21 Wy (row 0, unscaled: y is not doubled), 22/23 l0 Whh,
    # per layer 1..4: WihA@(24+4(l-1)) WhhA WihB WhhB -> 24..39
    wyA = np.zeros((128, 128), np.float32)
    wyB = np.zeros((128, 128), np.float32)
    wyA[0] = _chunk(dec_Wih0, _PERM_A, sA)[:, 0]
    wyB[0] = _chunk(dec_Wih0, _PERM_B, sB)[:, 0]
    blocks += [wyA, wyB]

    def k64b(W, b, perm, rowscale):
        # dec Whh block with the gate bias folded into row 64 (the rhs
        # Htb carries a const-ones line at partition 64); bias is NOT
        # halved (it multiplies ones, not H=2h).
        blk_ = k64(W, perm, rowscale)
        blk_[64] = _chunk(b[:, None], perm, rowscale)[:, 0]
        return blk_

    blocks += [k64b(dec_Whh[0], dec_b[0], _PERM_A, sA),
               k64b(dec_Whh[0], dec_b[0], _PERM_B, sB)]
    for l in range(1, L):
        blocks += [k64(dec_Wih[l - 1], _PERM_A, sA),
                   k64b(dec_Whh[l], dec_b[l], _PERM_A, sA),
                   k64(dec_Wih[l - 1], _PERM_B, sB),
                   k64b(dec_Whh[l], dec_b[l], _PERM_B, sB)]
    # decoder bias blocks 40..44: rows 0:2 = [bA; bB] (in-psum scale, g x2)
    for l in range(L):
        bb = np.zeros((128, 128), np.float32)
        bb[0] = _chunk(dec_b[l][:, None], _PERM_A, sA)[:, 0]
        bb[1] = _chunk(dec_b[l][:, None], _PERM_B, sB)[:, 0]
        blocks.append(bb)
    lstmw = np.concatenate(blocks, axis=1)  # [128, 45*128]
    # fc block: col 45*128 holds lhsT [64,1] = (0.5*fc_W).T
    fccol = np.zeros((128, 64), np.float32)
    fccol[0:64, 0] = 0.5 * fc_W[0]
    # conv0(+avgpool) 16-tap stationary [16, 64]:
    # V[t=(ty,tx), ch] = 1/4 sum_{ry,rx in 0..1} W0[ch, ty-ry, tx-rx]
    c0 = np.zeros((128, 64), np.float32)
    for ty in range(4):
        for tx in range(4):
            acc = np.zeros(64, np.float32)
            for ry in range(2):
                for rx in range(2):
                    ky, kx = ty - ry, tx - rx
                    if 0 <= ky < 3 and 0 <= kx < 3:
                        acc += conv0_W[:, 0, ky, kx]
            c0[ty * 4 + tx] = acc / 4.0
    lstmw = np.concatenate([lstmw, fccol, c0], axis=1).astype(BF)  # [128, 5888]

    # ---- cnnw: bf16 [128, 7*6*64]: uniform K=128 tap-pair stationaries ----
    # block p 0-2: rows 0:64 = tap (dy=p-1, dx=-1), rows 64:128 = tap (dy, 0)
    # block p 3-5: rows 0:64 = tap (dy=p-4, dx=+1), rows 64:128 = 0
    # (rhs bottom half is z pre-shifted by +1 column)
    cb = []
    for i in range(CNN_LAYERS - 1):
        for p in range(6):
            blk = np.zeros((128, 64), np.float32)
            if p < 3:
                dy = p - 1
                blk[0:64] = convs_W[i, :, :, dy + 1, 0].T
                blk[64:128] = convs_W[i, :, :, dy + 1, 1].T
            else:
                dy = p - 4
                blk[0:64] = convs_W[i, :, :, dy + 1, 2].T
            cb.append(blk)
    cnnw = np.concatenate(cb, axis=1).astype(BF)  # [128, 2688]

    # ---- misc: f32 [128, 32] ----
    misc = np.zeros((128, 32), np.float32)
    # decoder ACT bias (post-scale): i,f,o: 0.5*b ; g: b   (chunk-permuted)
    half = np.ones(256, np.float32) * 0.5
    half[128:192] = 1.0  # g rows (pytorch order) get 1.0
    for l in range(L):
        bb = dec_b[l] * half
        misc[:, 2 * l] = bb[_PERM_A]
        misc[:, 2 * l + 1] = bb[_PERM_B]
    misc[0, 10] = fc_b[0]
    misc[0:64, 11] = conv0_b
    for i in range(CNN_LAYERS - 1):
        misc[0:64, 12 + i] = convs_b[i]

    # ---- per-core tensors ----
    ypad = np.pad(y[:, 0], ((0, 0), (1, 1), (1, 1)))  # [B, 34, 34]
    in_maps = []
    for c in range(NCORES):
        sl = slice(c * BP, (c + 1) * BP)
        xs = x[sl, :, 0]  # [BP, T]
        # x2t [2, T*16 + 32]: row0 = x (time-major) then decoder-ones row0,
        # row1 = const ones (bias rhs line) then decoder-ones row1; the
        # decoder ones pattern covers both pipes (A|B per 16-col half)
        x2t = np.zeros((2, T * BP + 4 * BP), np.float32)
        x2t[0, 0:T * BP] = np.ascontiguousarray(xs.T).reshape(T * BP)
        x2t[1, 0:T * BP] = 1.0
        for po in (0, 16):
            x2t[0, T * BP + po:T * BP + po + 8] = 1.0
            x2t[1, T * BP + po + 8:T * BP + po + 16] = 1.0
        # single-pipe decoder ones pattern: [2, 32] (A cols 0:16, B 16:32)
        x2t[0, T * BP + 32:T * BP + 48] = 1.0
        x2t[1, T * BP + 48:T * BP + 64] = 1.0
        x2t = x2t.astype(BF)
        # pooled-conv0 16-tap im2col [16, BP*256]
        yp = ypad[sl]  # [BP, 34, 34]
        yim = np.zeros((16, BP, PM, PM), np.float32)
        for ty in range(4):
            for tx in range(4):
                yim[ty * 4 + tx] = yp[:, ty:ty + 2 * PM:2, tx:tx + 2 * PM:2]
        yim = yim.reshape(16, BP * PM * PM).astype(BF)
        in_maps.append(dict(
            lstmw=lstmw, cnnw=cnnw, misc=misc,
            x=x2t, yim=yim,
        ))
    return in_maps


# ----------------------------------------------------------------------------
# device program
# ----------------------------------------------------------------------------

_CACHE = {}


def build_program():
    import concourse.bass as bass  # noqa: F401
    import concourse.tile as tile
    from concourse import bacc, mybir

    F32 = mybir.dt.float32
    F32R = mybir.dt.float32r
    BF16 = mybir.dt.bfloat16
    AF = mybir.ActivationFunctionType
    OP = mybir.AluOpType

    KR = 4                       # ring depth: timesteps per Wih/bias batch
    TICKS = T + (L - 1) * KR     # 272
    DSTEPS = int(os.environ.get("BASSK_DSTEPS", PS))
    # NOTE: decoder matmul pre-accumulation (DPIPE=1) is numerically broken:
    # a matmul with start=True clears the has_written accumulation bits for
    # its ENTIRE psum bank (verified on HW), wiping other layers' open
    # accumulation groups that share the bank. Keep 0.
    HSPLIT = int(os.environ.get("BASSK_HSPLIT", 0))
    RELU_ACT = int(os.environ.get("BASSK_RELU_ACT", 1))
    DO_CNN = int(os.environ.get("BASSK_CNN", 1))
    NCONV = int(os.environ.get("BASSK_NCONV", CNN_LAYERS))
    DO_GAP = int(os.environ.get("BASSK_GAP", 1))

    nc = bacc.Bacc("TRN2", target_bir_lowering=False, debug=False,
                   num_devices=NCORES)
    d_lstmw = nc.dram_tensor("lstmw", [128, 5888], BF16, kind="ExternalInput").ap()
    d_cnnw = nc.dram_tensor("cnnw", [128, 2688], BF16, kind="ExternalInput").ap()
    d_misc = nc.dram_tensor("misc", [128, 32], F32, kind="ExternalInput").ap()
    d_x = nc.dram_tensor("x", [2, T * BP + 4 * BP], BF16,
                         kind="ExternalInput").ap()
    d_yim = nc.dram_tensor("yim", [16, BP * PM * PM], BF16,
                           kind="ExternalInput").ap()
    d_out = nc.dram_tensor("out", [1, PS * BP], F32, kind="ExternalOutput").ap()

    # stationary block column offsets in lstmw
    def blk(i):
        return slice(i * 128, (i + 1) * 128)
    FC_COL = 45 * 128
    C0_COL = 45 * 128 + 64
    KB = KR * BP                 # 64: ring cols per layer

    with tile.TileContext(nc) as tc:
        with ExitStack() as ctx:
            const = ctx.enter_context(tc.tile_pool(name="const", bufs=1))
            state = ctx.enter_context(tc.tile_pool(name="state", bufs=1))
            spool = ctx.enter_context(tc.tile_pool(name="spool", bufs=2))
            mpool = ctx.enter_context(tc.tile_pool(name="mpool", bufs=2))
            dpool = ctx.enter_context(tc.tile_pool(name="dpool", bufs=2))
            rps = ctx.enter_context(tc.tile_pool(name="rps", bufs=1, space="PSUM"))
            cps = ctx.enter_context(tc.tile_pool(name="cps", bufs=2, space="PSUM"))
            dps = ctx.enter_context(tc.tile_pool(name="dps", bufs=1, space="PSUM"))

            # ---- constants ----
            # DMA order: tick-0-critical pieces first (x line + the 20
            # encoder weight blocks), then the rest of lw, then CNN data,
            # so the encoder wavefront starts ~4us earlier.
            xw = const.tile([2, T * BP + 4 * BP], BF16, tag="xw", name="xw")
            nc.sync.dma_start(xw[:], d_x)
            misct = const.tile([128, 32], F32, tag="misct", name="misct")
            nc.sync.dma_start(misct[:], d_misc)
            lw = const.tile([128, 5888], BF16, tag="lw", name="lw")
            ENC_W = 20 * 128
            nc.sync.dma_start(lw[:, 0:ENC_W], d_lstmw[:, 0:ENC_W])
            nc.sync.dma_start(lw[:, ENC_W:], d_lstmw[:, ENC_W:])
            cw = const.tile([128, 2688], BF16, tag="cw", name="cw") if DO_CNN else None
            if DO_CNN:
                nc.sync.dma_start(cw[:], d_cnnw)
            yimt = const.tile([16, BP * PM * PM], BF16, tag="yimt", name="yimt") if DO_CNN else None
            if DO_CNN:
                nc.sync.dma_start(yimt[:], d_yim)

            # ---- persistent state ----
            # Hring[0:64, l*KB + j*BP + b] = 2*h^l[b] at timestep t, j=t%KR
            # (a KR-deep history ring per layer; row 64 = const ones so the
            # K=65 Wih-batch matmuls add the layer bias for free)
            Hring = state.tile([65, L * KB], BF16, tag="H", name="H")
            # Htb: decoder-only latest-H, two step-parity column banks;
            # partition 64 is a const-ones line so the K=65 Whh matmuls
            # add the decoder gate bias for free
            Htb = state.tile([65, 2 * W5], BF16, tag="Hd", name="Hd")
            nc.gpsimd.memset(Htb[64:65, :], 1.0)
            Ct = state.tile([64, W5], F32, tag="C", name="C")
            ydata = state.tile([1, BP], BF16, tag="ydata", name="ydata")
            nc.gpsimd.memset(Hring[0:64, :], 0.0)
            nc.gpsimd.memset(Hring[64:65, :], 1.0)
            nc.gpsimd.memset(Ct[:], 0.0)
            z2a = state.tile([128, BP * PIMG], BF16, tag="z2a", name="z2a") if DO_CNN else None
            z2b = state.tile([128, BP * PIMG], BF16, tag="z2b", name="z2b") if DO_CNN else None
            if DO_CNN:
                nc.gpsimd.memset(z2a[:], 0.0)
                nc.gpsimd.memset(z2b[:], 0.0)
            feat = state.tile([64, BP], F32, tag="feat", name="feat")
            feat2 = state.tile([64, BP], BF16, tag="feat2", name="feat2")
            outt = state.tile([1, PS * BP], F32, tag="outt", name="outt")
            if DSTEPS == 0:
                nc.gpsimd.memset(outt[:], 0.0)

            # =============== CNN emission slices ===============
            # CNN ops are emitted interleaved into the encoder tick loop in
            # small slices so PE/Scalar/Vector FIFO insertions never stall
            # the encoder's serial chain by more than ~1 op.
            cnn_slices = []
            if DO_CNN:
                c0st = lw[:, C0_COL:C0_COL + 64]  # [9 rows used, 64]
                z1v = z2a[:].rearrange("p (i r c) -> p i r c", i=BP, r=PPAD)

                def conv0_chunk(n):  # 2 pooled images per chunk, n = 0..7
                    def emit():
                        pc = cps.tile([64, 512], F32, tag="cpg", name="cpg")
                        nc.tensor.matmul(
                            pc[:], c0st[0:16, :],
                            yimt[0:16, n * 512:(n + 1) * 512],
                            start=True, stop=True)
                        i0 = 2 * n
                        pcv = pc[:].rearrange("p (i r c) -> p i r c",
                                              i=2, r=PM)
                        nc.scalar.activation(
                            z1v[0:64, i0:i0 + 2, 1:17, 1:17], pcv,
                            AF.Identity, bias=misct[0:64, 11:12])
                        nc.gpsimd.tensor_copy(
                            z1v[64:128, i0:i0 + 2, 1:17, 0:16],
                            z1v[0:64, i0:i0 + 2, 1:17, 1:17])
                    return emit

                for n in range(BP // 2):
                    cnn_slices.append(conv0_chunk(n))

                ccell = {}

                def conv_mms(i, n, prange, zin):
                    def emit():
                        ziv = zin[:].rearrange("p (i r c) -> p i r c",
                                               i=BP, r=PPAD)
                        if prange[0] == 0:
                            ccell['pc'] = cps.tile([64, 512], F32, tag="cpg",
                                                   name="cpg")
                        pc = ccell['pc']
                        i0 = 2 * n
                        for p in prange:
                            dy = (p - 1) if p < 3 else (p - 4)
                            c0_ = 0 if p < 3 else 2
                            st_ = cw[:, (i - 1) * 384 + p * 64:
                                     (i - 1) * 384 + p * 64 + 64]
                            rhs = ziv[:, i0:i0 + 2, 1 + dy:17 + dy,
                                      c0_:c0_ + 16]
                            nc.tensor.matmul(pc[:], st_, rhs,
                                             start=(p == 0), stop=(p == 5))
                    return emit

                def conv_relu(i, n, half, zout):
                    def emit():
                        pc = ccell['pc']
                        im = 2 * n + half
                        zov = zout[:].rearrange("p (i r c) -> p i r c",
                                                i=BP, r=PPAD)
                        pcv = pc[:].rearrange("p (i r c) -> p i r c",
                                              i=2, r=16)[:, half]
                        if i < CNN_LAYERS - 1:
                            # relu on Scalar ACT (measured best; a 50/50
                            # Scalar/DVE alternation and all-DVE were both
                            # slower on this base)
                            if RELU_ACT:
                                nc.scalar.activation(
                                    zov[0:64, im, 1:17, 1:17], pcv,
                                    AF.Relu,
                                    bias=misct[0:64, 11 + i:12 + i])
                            else:
                                nc.vector.tensor_scalar(
                                    zov[0:64, im, 1:17, 1:17], pcv,
                                    misct[0:64, 11 + i:12 + i], 0.0,
                                    op0=OP.add, op1=OP.max)
                            nc.gpsimd.tensor_copy(
                                zov[64:128, im, 1:17, 0:16],
                                zov[0:64, im, 1:17, 1:17])
                        else:
                            # last conv: Scalar ACT whose accum_out side
                            # output IS the global-avg-pool sum -- removes
                            # the relu AND the GAP reduction from DVE
                            nc.scalar.activation(
                                zov[0:64, im, 1:17, 1:17], pcv,
                                AF.Relu, bias=misct[0:64, 11 + i:12 + i],
                                accum_out=feat[:, im:im + 1])
                    return emit

                zin, zout = z2a, z2b
                for i in range(1, NCONV):
                    for n in range(BP // 2):
                        for p in range(6):
                            cnn_slices.append(conv_mms(i, n, (p,), zin))
                        cnn_slices.append(conv_relu(i, n, 0, zout))
                        cnn_slices.append(conv_relu(i, n, 1, zout))
                    zin, zout = zout, zin

                if DO_GAP:
                    # GAP already accumulated by the last conv layer's ACT
                    # accum_out; just cast to bf16 for the fuse
                    cnn_slices.append(
                        lambda: nc.vector.tensor_copy(feat2[:], feat[:]))
                else:
                    cnn_slices.append(
                        lambda: nc.gpsimd.memset(feat2[:], 0.0))
            else:
                cnn_slices.append(lambda: nc.gpsimd.memset(feat2[:], 0.0))

            # =============== decoder emission closures ===============
            # Defined up front so step 0's cells 0..3 can be emitted
            # INTERLEAVED into the encoder's tail ticks (layer l's final
            # encoder tick is 255+KR*l, so fuse-l and decoder cell l can
            # start while upper layers are still finishing).
            kf = 2.0 * ALPHA / 256.0
            jfin = (T - 1) % KR
            HB = BP // 2
            ones8 = xw[0:2, T * BP:T * BP + 2 * HB]

            def emit_fuse(l):
                # initial decoder state -> parity-1 columns of Htb
                nc.vector.scalar_tensor_tensor(
                    Htb[0:64, W5 + l * BP:W5 + (l + 1) * BP], feat2[:], kf,
                    Hring[0:64, l * KB + jfin * BP:l * KB + (jfin + 1) * BP],
                    op0=OP.mult, op1=OP.add)
                if l == 0:
                    nc.vector.tensor_copy(ydata[0:1, :],
                                          xw[0:1, (T - 1) * BP:T * BP])

            ones16 = xw[0:2, T * BP + 32:T * BP + 64]

            def emit_cell(step, l):
                # single 16-batch pipe: half the ops of the 2-pipe form
                # (1 gate ACT, 3 STT, 1 tc, 1 H per cell) -- bets queue
                # relief over pipe overlap (Scalar was ~70% busy)
                par = step % 2
                hp = (step - 1) % 2
                whA = blk(22) if l == 0 else blk(24 + 4 * (l - 1) + 1)
                whB = blk(23) if l == 0 else blk(24 + 4 * (l - 1) + 3)
                ch = hp * W5 + l * BP
                if l == 0 and 'pd' in pdz:
                    # off-path group pre-emitted before the previous fc
                    pd = pdz.pop('pd')
                else:
                    pd = dps.tile([128, 2 * BP], F32, tag="dpg0",
                                  name="dpg0")
                    nc.tensor.matmul(pd[:, 0:2 * BP], lw[0:2, blk(40 + l)],
                                     ones16, start=True, stop=False)
                    nc.tensor.matmul(pd[:, 0:BP], lw[0:64, whA],
                                     Htb[0:64, ch:ch + BP],
                                     start=False, stop=False)
                    nc.tensor.matmul(pd[:, BP:2 * BP], lw[0:64, whB],
                                     Htb[0:64, ch:ch + BP],
                                     start=False, stop=False)
                if l == 0:
                    nc.tensor.matmul(pd[:, 0:BP], lw[0:1, blk(20)],
                                     ydata[0:1, :], start=False, stop=True)
                    nc.tensor.matmul(pd[:, BP:2 * BP], lw[0:1, blk(21)],
                                     ydata[0:1, :], start=False, stop=True)
                else:
                    b0 = 24 + 4 * (l - 1)
                    p0 = par * W5 + (l - 1) * BP
                    nc.tensor.matmul(pd[:, 0:BP], lw[0:64, blk(b0)],
                                     Htb[0:64, p0:p0 + BP],
                                     start=False, stop=True)
                    nc.tensor.matmul(pd[:, BP:2 * BP], lw[0:64, blk(b0 + 2)],
                                     Htb[0:64, p0:p0 + BP],
                                     start=False, stop=True)
                sd = dpool.tile([128, 2 * BP], BF16, tag="sdec0",
                                name="sdec0")
                nc.scalar.activation(sd[:], pd[:], AF.Tanh, scale=0.5)
                c0 = l * BP
                dm1 = mpool.tile([64, BP], F32, tag="dm10", name="dm10")
                dm2 = mpool.tile([64, BP], BF16, tag="dm20", name="dm20")
                nc.vector.scalar_tensor_tensor(
                    dm1[:], sd[0:64, 0:BP], 1.0, Ct[:, c0:c0 + BP],
                    op0=OP.add, op1=OP.mult)
                nc.vector.scalar_tensor_tensor(
                    dm2[:], sd[64:128, 0:BP], 1.0, sd[64:128, BP:2 * BP],
                    op0=OP.add, op1=OP.mult)
                nc.vector.scalar_tensor_tensor(
                    Ct[:, c0:c0 + BP], dm1[:], 0.5, dm2[:],
                    op0=OP.mult, op1=OP.add)
                dtc = mpool.tile([64, BP], BF16, tag="dtc0", name="dtc0")
                nc.scalar.activation(dtc[:], Ct[:, c0:c0 + BP],
                                     AF.Tanh, scale=0.5)
                nc.vector.scalar_tensor_tensor(
                    Htb[0:64, par * W5 + c0:par * W5 + c0 + BP],
                    sd[0:64, BP:2 * BP], 1.0, dtc[:],
                    op0=OP.add, op1=OP.mult)

            pdz = {}

            def emit_fc(step):
                par = step % 2
                if step + 1 < DSTEPS:
                    # next step's cell-0 bias+Whh: inputs are ready (h^0 of
                    # this step), dedicated bank dpgZ has no open group
                    pdn = dps.tile([128, 2 * BP], F32, tag="dpgZ",
                                   name="dpgZ")
                    chn = par * W5
                    nc.tensor.matmul(pdn[:, 0:2 * BP], lw[0:2, blk(40)],
                                     ones16, start=True, stop=False)
                    nc.tensor.matmul(pdn[:, 0:BP], lw[0:64, blk(22)],
                                     Htb[0:64, chn:chn + BP],
                                     start=False, stop=False)
                    nc.tensor.matmul(pdn[:, BP:2 * BP], lw[0:64, blk(23)],
                                     Htb[0:64, chn:chn + BP],
                                     start=False, stop=False)
                    pdz['pd'] = pdn
                pf = dps.tile([128, BP], F32, tag="pfc0", name="pfc0")
                nc.tensor.matmul(
                    pf[0:1, 0:BP], lw[0:64, FC_COL:FC_COL + 1],
                    Htb[0:64, par * W5 + (L - 1) * BP:par * W5 + L * BP],
                    start=True, stop=True)
                if step + 1 < DSTEPS:
                    nc.vector.tensor_scalar(
                        ydata[0:1, :], pf[0:1, 0:BP],
                        misct[0:1, 10:11], 0.0, op0=OP.add, op1=OP.add)
                nc.vector.tensor_scalar(
                    outt[0:1, step * BP:(step + 1) * BP], pf[0:1, 0:BP],
                    misct[0:1, 10:11], 0.0, op0=OP.add, op1=OP.add)

            # =============== encoder wavefront (psum ring) ===============
            # Persistent psum rings [128 gates, L layers x KR slots x BP]:
            # every KR ticks per layer, one K=65 batch matmul (start=True)
            # writes Wih@h^{l-1} + bias for KR future steps; each tick only
            # the serial Whh matmul (stop=True, slot t%KR) is on the chain.
            pgA = rps.tile([128, L * KB], F32, tag="pgA", name="pgA")
            pgB = rps.tile([128, L * KB], F32, tag="pgB", name="pgB")
            pgA4 = pgA[:].rearrange("p (l j b) -> p l j b", l=L, j=KR)
            pgB4 = pgB[:].rearrange("p (l j b) -> p l j b", l=L, j=KR)
            Hr4 = Hring[0:64, :].rearrange("p (l j b) -> p l j b", l=L, j=KR)

            cnn_emitted = 0
            for s in range(TICKS):
                lmax = min(L - 1, s // KR)
                lmin = max(0, -((-(s - (T - 1))) // KR))
                lo, w = lmin * BP, (lmax - lmin + 1) * BP
                nl = lmax - lmin + 1
                j = s % KR
                jp = (s - 1) % KR

                # group-start batch MMs (Wih+bias over KR steps, off-chain)
                for l in range(lmin, lmax + 1):
                    t = s - KR * l
                    if t % KR == 0:
                        for c, pgc in ((0, pgA), (1, pgB)):
                            if l == 0:
                                nc.tensor.matmul(
                                    pgc[:, 0:KB], lw[0:2, blk(c)],
                                    xw[0:2, t * BP:t * BP + KB],
                                    start=True, stop=False)
                            else:
                                nc.tensor.matmul(
                                    pgc[:, l * KB:(l + 1) * KB],
                                    lw[0:65, blk(12 + 2 * (l - 1) + c)],
                                    Hring[0:65, (l - 1) * KB:l * KB],
                                    start=True, stop=False)
                # per-tick serial Whh MMs (slot j), chunk A then B
                for c, pgc in ((0, pgA), (1, pgB)):
                    for l in range(lmin, lmax + 1):
                        nc.tensor.matmul(
                            pgc[:, l * KB + j * BP:l * KB + (j + 1) * BP],
                            lw[0:64, blk(2 + 2 * l + c)],
                            Hring[0:64, l * KB + jp * BP:l * KB + (jp + 1) * BP],
                            start=False, stop=True)

                # gates: per-chunk tanh so chunk-A consumers start earlier
                stA = spool.tile([128, W5], BF16, tag="stA", name="stA")
                stB = spool.tile([128, W5], BF16, tag="stB", name="stB")
                sAv = stA[:, lo:lo + w].rearrange("p (l b) -> p l b", b=BP)
                sBv = stB[:, lo:lo + w].rearrange("p (l b) -> p l b", b=BP)
                nc.scalar.activation(sAv, pgA4[:, lmin:lmax + 1, j, :],
                                     AF.Tanh, scale=0.5)
                nc.scalar.activation(sBv, pgB4[:, lmin:lmax + 1, j, :],
                                     AF.Tanh, scale=0.5)

                m1 = mpool.tile([64, W5], F32, tag="m1", name="m1")
                m2 = mpool.tile([64, W5], BF16, tag="m2", name="m2")
                tcn = mpool.tile([64, W5], BF16, tag="tc", name="tc")
                # m1 = (sf+1)*C  (needs only chunk A; runs under ACT-B)
                nc.vector.scalar_tensor_tensor(
                    m1[:, lo:lo + w], stA[0:64, lo:lo + w], 1.0,
                    Ct[:, lo:lo + w], op0=OP.add, op1=OP.mult)
                # m2 = (si+1)*sg
                nc.vector.scalar_tensor_tensor(
                    m2[:, lo:lo + w], stA[64:128, lo:lo + w], 1.0,
                    stB[64:128, lo:lo + w], op0=OP.add, op1=OP.mult)
                # C = 0.5*m1 + m2
                nc.vector.scalar_tensor_tensor(
                    Ct[:, lo:lo + w], m1[:, lo:lo + w], 0.5,
                    m2[:, lo:lo + w], op0=OP.mult, op1=OP.add)
                # tc = tanh(0.5*C)
                nc.scalar.activation(tcn[:, lo:lo + w], Ct[:, lo:lo + w],
                                     AF.Tanh, scale=0.5)
                # H = (so+1)*tc -> ring slot j of each active layer.
                # Split low/high layers so the next tick's low-layer Whh
                # matmuls can issue ~200ns earlier (the loop is the bound).
                lsp = min(lmax, max(lmin, 2)) if HSPLIT else lmax
                for (g0, g1) in (((lmin, lsp)), ((lsp + 1, lmax))):
                    if g1 < g0:
                        continue
                    c0_, c1_ = g0 * BP, (g1 + 1) * BP
                    nc.vector.scalar_tensor_tensor(
                        Hr4[:, g0:g1 + 1, j, :],
                        stB[0:64, c0_:c1_].rearrange("p (l b) -> p l b", b=BP),
                        1.0,
                        tcn[:, c0_:c1_].rearrange("p (l b) -> p l b", b=BP),
                        op0=OP.add, op1=OP.mult)

                # pace CNN slices so they finish by tick ~250 (feat2 must
                # be ready before fuse-l0 fires at tick 255), queued behind
                # this tick's ops to land in the idle windows
                target = min(len(cnn_slices),
                             (s + 1) * len(cnn_slices) // min(TICKS, 250))
                while cnn_emitted < target:
                    cnn_slices[cnn_emitted]()
                    cnn_emitted += 1

                # overlap decoder start with the encoder tail: fuse-l right
                # after layer l's last tick; step-0 cell l one tick later
                if DSTEPS > 0 and s >= T - 1 and (s - (T - 1)) % KR == 0:
                    lf = (s - (T - 1)) // KR
                    if lf < L:
                        emit_fuse(lf)
                if DSTEPS > 0 and s >= T and (s - T) % KR == 0:
                    lc = (s - T) // KR
                    if lc < L - 1:
                        emit_cell(0, lc)

            # leftover CNN slices (if any) after the encoder
            for k in range(cnn_emitted, len(cnn_slices)):
                cnn_slices[k]()

            # =============== decoder (two half-batch pipes) ===============
            # Two independent 8-batch chains interleave on the engines so
            # each pipe's serial latency hides under the other's work.
            # Step 0 cells 0..L-2 were already emitted in the tail ticks.
            for step in range(DSTEPS):
                for l in range(L - 1 if step == 0 else 0, L):
                    emit_cell(step, l)
                emit_fc(step)

            nc.sync.dma_start(d_out, outt[:])

    nc.compile()
    return nc


def kernel(**inputs) -> np.ndarray:
    from concourse.bass_utils import run_bass_kernel_spmd
    if "nc" not in _CACHE:
        _CACHE["nc"] = build_program()
    nc = _CACHE["nc"]
    in_maps = prep_host(inputs)
    res = run_bass_kernel_spmd(nc, in_maps, list(range(NCORES)))
    outs = []
    for c in range(NCORES):
        o = np.asarray(res.results[c]["out"], np.float32).reshape(PS, BP)
        outs.append(o.T[:, :, None])  # [BP, PS, 1]
    return np.concatenate(outs, axis=0)

